# revision 13
# baseline (speedup 1.0000x reference)
"""Mesa-layer memory kernel for Trainium2 (8 NeuronCores, data-parallel over B).

Math: the reference's T-step Sherman-Morrison / discounted-accumulation
recurrence has the closed form
    R_final = (I + K^T K)^{-1}            (eps term is O(1e-6) relative)
    S_final^T = K^T diag(c) V,   c_t = prod_{s>t} gamma_s
so per memory b the output is out_b = Q_b @ (R_b @ S_b^T).

R comes from Newton-Schulz in residual form  X <- X + X^T (I - A X),
3 bf16 iterations + 1 refinement whose residual is computed in fp32 and
rounded to bf16 (X0 = 1.5 * diag(1/rowsum_abs(A)) centers the spectrum;
converges to the bf16 noise floor, ~5.8e-3 max-rel).

Schedule notes (what actually mattered on HW):
  - ALL inputs stream through one SWDGE (gpsimd) ring that casts
    fp32->bf16 inline; ring FIFO order IS the priority order:
    gamma, K/V group 0, Q0-1, K/V group 1, Q2-7.  Pool-buffer waits pace
    descriptor generation so emission order survives to the wire.
  - gammas are pre-transposed on the HOST to [P, B', R16] so the gamma
    DMA is one contiguous 64 KB transfer (the natural layout needs 1024
    64 B descriptors, which round-robin against 8 KB packets and take
    20+ us to drain).
  - The first input DMAs are emitted BEFORE mask building (masks run on
    GpSimd and would block SWDGE descriptor generation ~10 us).  Only
    ident/utri are built with GpSimd; derived constants copy on DVE.
  - ~64 dummy matmuls at the head of the PE queue warm the HAM clock
    gate (cold PE runs at 1.2 GHz = 2x matmul cost) before the first
    K arrives, and double as the identity-constant pipeline warmer.
  - A = I + K^T K and S^T = K^T (cV) accumulate in SEPARATE PSUM banks
    (a PSUM accumulation group owns its whole 2 KB zero region).
  - c is expanded to [P,R16,DV] by a broadcast-input activation copy on
    ScalarE and applied to V as a flat bf16 DVE multiply (the only
    engine/shape pair that dodges the DVE broadcast slow path).
  - Readout: phi^T stationary, out^T = phi^T.T @ Q^T via 16 PE
    transposes + 4 N=512 matmuls per memory; output is written
    TRANSPOSED as fp16 [DV, (r,p)] on the scalar HWDGE ring and
    unpermuted on the host.  All 8 Q tiles are resident (bf16) so the
    readout tail never waits on a buffer.
  - Group 0's NS stages interleave with group 1's A/S chains, group 1's
    with group 0's readouts; the strict-FIFO engines pace each NS stage
    by the interleaved work, so that work is kept small and warm.

Layout: timestep t maps to (partition p, slot r) via t = 16 p + r, making
every HBM transfer a fully contiguous 1 MB read.
"""

import numpy as np

B, T, DK, DV, NQ = 64, 2048, 128, 128, 2048
NCORES = 8
BPC = B // NCORES          # memories per core
P = 128                    # partitions
R16 = T // P               # 16 row-slots per partition
GCLAMP = 1e-30             # gamma clamp before log (exact-0 gammas)

NS_BF = 3                  # Newton-Schulz iterations in bf16
NS_FP = 1                  # refinement iterations (fp32 residual)
X0C = 1.0 / (T + DK + 1)   # X0 = c*I; c = 2/(lmin+lmax) for Wishart A
NWARM = 64                 # PE warm-up matmuls


def build_nc(ns_bf=NS_BF, ns_fp=NS_FP):
    import concourse.mybir as mybir
    import concourse.tile as tile
    from concourse import bacc

    fp32 = mybir.dt.float32
    bf16 = mybir.dt.bfloat16
    fp16 = mybir.dt.float16
    AF = mybir.ActivationFunctionType
    OP = mybir.AluOpType
    AX = mybir.AxisListType
    NIT = ns_bf + ns_fp

    nc = bacc.Bacc(trn_type="TRN2", target_bir_lowering=False, debug=False)
    keys = nc.dram_tensor("keys", [BPC, T, DK], fp32, kind="ExternalInput").ap()
    values = nc.dram_tensor("values", [BPC, T, DV], fp32, kind="ExternalInput").ap()
    gammas = nc.dram_tensor("gammas", [P, BPC, R16], fp32, kind="ExternalInput").ap()
    queries = nc.dram_tensor("queries", [BPC, NQ, DK], fp32, kind="ExternalInput").ap()
    c_ident = nc.dram_tensor("c_ident", [P, P], fp32, kind="ExternalInput").ap()
    c_identbf = nc.dram_tensor("c_identbf", [P, P], bf16, kind="ExternalInput").ap()
    c_ident4 = nc.dram_tensor("c_ident4", [P, 4 * P], fp32, kind="ExternalInput").ap()
    c_utri = nc.dram_tensor("c_utri", [P, P], fp32, kind="ExternalInput").ap()
    c_ones = nc.dram_tensor("c_ones", [P, P], fp32, kind="ExternalInput").ap()
    # transposed fp16 output: out_t[i, v, r*P + p] = out[i, 16p + r, v]
    out = nc.dram_tensor("out", [BPC, DV, NQ], fp16, kind="ExternalOutput").ap()

    with tile.TileContext(nc) as tc:
        const = tc.alloc_tile_pool(name="const", bufs=1)
        gam = tc.alloc_tile_pool(name="gam", bufs=1)
        kp = tc.alloc_tile_pool(name="kp", bufs=8)
        vp = tc.alloc_tile_pool(name="vp", bufs=4)
        cvp = tc.alloc_tile_pool(name="cvp", bufs=2)
        cep = tc.alloc_tile_pool(name="cep", bufs=8)
        qbp = tc.alloc_tile_pool(name="qbp", bufs=8)
        qtp = tc.alloc_tile_pool(name="qtp", bufs=3)
        small = tc.alloc_tile_pool(name="small", bufs=1)
        xs = tc.alloc_tile_pool(name="xs", bufs=2)
        xf = tc.alloc_tile_pool(name="xf", bufs=1)
        outp = tc.alloc_tile_pool(name="outp", bufs=2)
        ps_a = tc.alloc_tile_pool(name="ps_a", bufs=2, space="PSUM")
        ps_s = tc.alloc_tile_pool(name="ps_s", bufs=2, space="PSUM")
        ps_w = tc.alloc_tile_pool(name="ps_w", bufs=4, space="PSUM")

        NGRP = 2
        GSZ = BPC // NGRP

        # ---- input streams first: SWDGE ring order = priority order ----
        ident = const.tile([P, P], fp32)
        nc.gpsimd.dma_start(ident[:], c_ident)
        ident_bf = const.tile([P, P], bf16)
        nc.gpsimd.dma_start(ident_bf[:], c_identbf)
        ident4 = const.tile([P, 4 * P], fp32)
        nc.gpsimd.dma_start(ident4[:], c_ident4)
        utri = const.tile([P, P], fp32)
        nc.gpsimd.dma_start(utri[:], c_utri)
        ones2 = const.tile([P, P], fp32)
        nc.gpsimd.dma_start(ones2[:], c_ones)
        g16 = gam.tile([P, BPC, R16], fp32)
        nc.gpsimd.dma_start(g16[:], gammas)

        kt = [None] * BPC
        vbt = [None] * BPC
        cvt = [None] * BPC
        Qb = [None] * BPC

        def k_dma(i):
            kt[i] = kp.tile([P, R16, DK], bf16, tag="k", name=f"k{i}")
            nc.gpsimd.dma_start(kt[i][:], keys[i].rearrange("(p r) k -> p r k", p=P))

        def v_dma(i):
            vbt[i] = vp.tile([P, R16, DV], bf16, tag="v", name=f"v{i}")
            nc.gpsimd.dma_start(
                vbt[i][:], values[i].rearrange("(p r) k -> p r k", p=P)
            )

        def load_q(i):
            Qb[i] = qbp.tile([P, R16, DK], bf16, tag="qb", name=f"qb{i}")
            nc.gpsimd.dma_start(
                Qb[i][:], queries[i].rearrange("(p r) k -> p r k", p=P)
            )

        # K first (the NS critical path), then V group 0, Q0-1, V group 1,
        # Q2-7 -- pool-buffer waits keep this emission order on the wire
        for i in range(BPC):
            k_dma(i)
        for i in range(GSZ):
            v_dma(i)
        load_q(0)
        load_q(1)
        for i in range(GSZ, BPC):
            v_dma(i)
        for i in range(2, BPC):
            load_q(i)

        # ---- PE warm-up: dummy matmuls keep the HAM clock gate open ----
        dummy = const.tile([P, P], bf16)
        nc.vector.memset(dummy[:], 0.25)
        for w in range(NWARM):
            dps = ps_w.tile([P, P], fp32, tag="w", name=f"warm{w}")
            nc.tensor.matmul(dps[:], dummy[:], dummy[:])

        # ---- phase 0: suffix cumprod of gammas (log space) ----
        g16f = g16.rearrange("p i r -> p (i r)")
        nc.vector.tensor_scalar_max(g16f, g16f, GCLAMP)
        nc.scalar.activation(g16f, g16f, AF.Ln)
        incl = gam.tile([P, BPC, R16], fp32)
        zz = gam.tile([P, R16], fp32)
        nc.vector.memset(zz[:], 0.0)
        for i in range(BPC):
            nc.vector.tensor_tensor_scan(
                incl[:, i, :], g16[:, i, :], zz[:], 0.0, OP.add, OP.add
            )
        ptot = gam.tile([P, BPC], fp32)
        nc.vector.tensor_copy(out=ptot[:], in_=incl[:, :, R16 - 1])
        ps_pre = ps_w.tile([P, 2 * BPC], fp32, tag="w", name="ps_pre")
        nc.tensor.matmul(ps_pre[:, 0:BPC], utri[:], ptot[:])
        nc.tensor.matmul(ps_pre[:, BPC : 2 * BPC], ones2[:], ptot[:])
        pre_sb = gam.tile([P, 2 * BPC], fp32)
        nc.vector.tensor_copy(out=pre_sb[:], in_=ps_pre[:])
        bias2 = gam.tile([P, BPC], fp32)
        nc.vector.tensor_tensor(
            bias2[:], pre_sb[:, BPC : 2 * BPC], pre_sb[:, 0:BPC], OP.subtract
        )
        # c_t[p, i, r] = exp(bias - incl) = prod_{s > 16p+r} gamma[i, s]
        c_t = gam.tile([P, BPC, R16], fp32)
        for i in range(BPC):
            nc.scalar.activation(
                c_t[:, i, :], incl[:, i, :], AF.Exp,
                bias=bias2[:, i : i + 1], scale=-1.0,
            )

        # ---- per-memory state tiles ----
        A_sb = [small.tile([P, P], fp32, tag=f"A{i}", name=f"A{i}") for i in range(BPC)]
        A_bf = [small.tile([P, P], bf16, tag=f"Ab{i}", name=f"Ab{i}") for i in range(BPC)]
        ST_sb = [small.tile([P, P], fp32, tag=f"S{i}", name=f"S{i}") for i in range(BPC)]
        Phi_bf = [small.tile([P, P], bf16, tag=f"Pb{i}", name=f"Phib{i}") for i in range(BPC)]

        cet = [None] * BPC

        def make_ce(i):
            """expand c to [P,R16,DV] on ScalarE (broadcast-input copy)."""
            cet[i] = cep.tile([P, R16, DV], bf16, tag="ce", name=f"ce{i}")
            nc.scalar.activation(
                cet[i][:], c_t[:, i, :, None].to_broadcast((P, R16, DV)), AF.Copy
            )

        def v_use(i):
            """cv = v * ce (flat bf16 DVE multiply)."""
            cvt[i] = cvp.tile([P, R16, DV], bf16, tag="cv", name=f"cv{i}")
            nc.vector.tensor_tensor(cvt[i][:], vbt[i][:], cet[i][:], OP.mult)

        def chain_a(i):
            """A = I + K^T K accumulated in its own PSUM bank."""
            psa = ps_a.tile([P, 4 * P], fp32, tag="a", name=f"ps_a{i}")
            for r in range(R16):
                nc.tensor.matmul(psa[:, 0:P], kt[i][:, r, :], kt[i][:, r, :],
                                 start=(r == 0), stop=(r == R16 - 1))
            nc.vector.tensor_tensor(A_sb[i][:], psa[:, 0:P], ident[:], OP.add)
            nc.scalar.copy(out=A_bf[i][:], in_=A_sb[i][:])

        def chain_s(i):
            """S^T = K^T (cV) accumulated in its own PSUM bank."""
            pss = ps_s.tile([P, 4 * P], fp32, tag="s", name=f"ps_s{i}")
            for r in range(R16):
                nc.tensor.matmul(pss[:, 0:P], kt[i][:, r, :], cvt[i][:, r, :],
                                 start=(r == 0), stop=(r == R16 - 1))
            nc.vector.tensor_copy(out=ST_sb[i][:], in_=pss[:, 0:P])

        Xg = [None] * NGRP

        def x0(g):
            xw = xs.tile([P, GSZ * P], bf16, tag=f"Xb{g}", name=f"Xb{g}_0")
            nc.scalar.activation(
                xw[:], ident4[:, 0 : GSZ * P], AF.Copy, scale=X0C
            )
            Xg[g] = xw

        def ns_stage(g, it):
            """One residual-form NS iteration for group g."""
            bf_iter = it < ns_bf
            Amat = A_bf if bf_iter else A_sb
            pa = ps_w.tile([P, GSZ * P], fp32, tag="w", name=f"pa{g}_{it}")
            for i in range(GSZ):
                sl = slice(i * P, (i + 1) * P)
                nc.tensor.matmul(pa[:, sl], Amat[GSZ * g + i][:], Xg[g][:, sl])
            eg = xs.tile([P, GSZ * P], bf16, tag=f"e{g}", name=f"e{g}_{it}")
            nc.vector.scalar_tensor_tensor(
                eg[:], pa[:], -1.0, ident4[:, 0 : GSZ * P], OP.mult, OP.add
            )
            if bf_iter:
                Xb = Xg[g]
            else:
                Xb = xs.tile([P, GSZ * P], bf16, tag=f"Xb{g}", name=f"Xbf{g}_{it}")
                nc.scalar.copy(out=Xb[:], in_=Xg[g][:])
            pb = ps_w.tile([P, GSZ * P], fp32, tag="w", name=f"pb{g}_{it}")
            for i in range(GSZ):
                sl = slice(i * P, (i + 1) * P)
                nc.tensor.matmul(pb[:, sl], Xb[:, sl], eg[:, sl])
            if it >= ns_bf - 1:
                xn = xf.tile([P, GSZ * P], fp32,
                             tag=f"Xf{g}_{it}", name=f"X{g}_{it + 1}")
            else:
                xn = xs.tile([P, GSZ * P], bf16, tag=f"Xb{g}", name=f"X{g}_{it + 1}")
            nc.vector.tensor_tensor(xn[:], Xg[g][:], pb[:], OP.add)
            Xg[g] = xn

        def phi(i):
            g, sl = i // GSZ, slice((i % GSZ) * P, (i % GSZ + 1) * P)
            ps_phi = ps_a.tile([P, 4 * P], fp32, tag="a", name=f"ps_phi{i}")
            nc.tensor.matmul(ps_phi[:, 0:P], Xg[g][:, sl], ST_sb[i][:])
            nc.scalar.copy(out=Phi_bf[i][:], in_=ps_phi[:, 0:P])

        def readout(i):
            """out^T[v, (r,p)] = phi^T.T @ Q^T, 4-slot batches."""
            o_sb = outp.tile([P, R16, P], fp16, tag="o", name=f"o{i}")
            for j in range(R16 // 4):
                pq = ps_w.tile([P, 4 * P], bf16, tag="w", name=f"ps_qt{i}_{j}")
                for r in range(4):
                    nc.tensor.transpose(
                        pq[:, r * P : (r + 1) * P], Qb[i][:, 4 * j + r, :],
                        ident_bf[:],
                    )
                qt = qtp.tile([P, 4 * P], bf16, tag="qt", name=f"qt{i}_{j}")
                if j % 2 == 0:
                    nc.vector.tensor_copy(out=qt[:], in_=pq[:])
                else:
                    nc.scalar.copy(out=qt[:], in_=pq[:])
                po = ps_w.tile([P, 4 * P], fp32, tag="w", name=f"ps_o{i}_{j}")
                nc.tensor.matmul(po[:], Phi_bf[i][:], qt[:])
                nc.scalar.copy(out=o_sb[:, 4 * j : 4 * j + 4, :], in_=po[:])
            nc.scalar.dma_start(
                out[i].rearrange("v (r p) -> v r p", r=R16), o_sb[:]
            )

        # ---- emission ----
        for i in range(BPC):
            make_ce(i)
        for i in range(GSZ):
            chain_a(i)
        x0(0)
        # group 0 NS; fillers: S-chains of group 0, A-chains of group 1
        for it in range(NIT):
            ns_stage(0, it)
            if it < GSZ:
                v_use(it)
                chain_s(it)
                chain_a(GSZ + it)
        for i in range(GSZ):
            phi(i)
        x0(1)
        # group 1 NS; fillers: S-chains of group 1, readouts of group 0
        for it in range(NIT):
            ns_stage(1, it)
            if it < GSZ:
                v_use(GSZ + it)
                chain_s(GSZ + it)
                readout(it)
        for i in range(GSZ, BPC):
            phi(i)
        for i in range(GSZ, BPC):
            readout(i)

        for pool in (ps_w, ps_s, ps_a, outp, xf, xs, small, qtp, qbp,
                     cep, cvp, vp, kp, gam, const):
            pool.release()

    if not nc.is_finalized():
        nc.finalize()
    return nc


def unshard_out(res_list):
    """[DV, (r,p)] fp16 per memory -> [B', NQ, DV] fp32."""
    o = np.concatenate(res_list, axis=0)          # [B', DV, NQ] fp16
    o = o.astype(np.float32).reshape(-1, DV, R16, P)
    return np.ascontiguousarray(o.transpose(0, 3, 2, 1)).reshape(-1, NQ, DV)


def make_in_maps(inputs):
    import ml_dtypes
    ident = np.eye(P, dtype=np.float32)
    consts = {
        "c_ident": ident,
        "c_identbf": ident.astype(ml_dtypes.bfloat16),
        "c_ident4": np.ascontiguousarray(np.tile(ident, (1, 4))),
        "c_utri": np.triu(np.ones((P, P), dtype=np.float32), k=1),
        "c_ones": np.ones((P, P), dtype=np.float32),
    }
    keys = np.ascontiguousarray(inputs["keys"], dtype=np.float32)
    values = np.ascontiguousarray(inputs["values"], dtype=np.float32)
    gammas = np.ascontiguousarray(inputs["gammas"], dtype=np.float32)
    queries = np.ascontiguousarray(inputs["queries"], dtype=np.float32)
    in_maps = []
    for m in range(NCORES):
        s = slice(m * BPC, (m + 1) * BPC)
        # gamma pre-layout: [B', T] -> [P, B', R16] with t = 16p + r, so the
        # device-side gamma DMA is one fully contiguous 64 KB transfer
        g = np.ascontiguousarray(
            gammas[s].reshape(BPC, P, R16).transpose(1, 0, 2)
        )
        in_maps.append(
            {
                "keys": keys[s],
                "values": values[s],
                "gammas": g,
                "queries": queries[s],
                **consts,
            }
        )
    return in_maps


def kernel(**inputs) -> np.ndarray:
    from concourse.bass_utils import run_bass_kernel_spmd

    nc = build_nc()
    res = run_bass_kernel_spmd(
        nc, make_in_maps(inputs), core_ids=list(range(NCORES))
    )
    return unshard_out([res.results[m]["out"] for m in range(NCORES)])
